# revision 13
# baseline (speedup 1.0000x reference)
"""Trainium2 Bass kernel for nn_Decoder_TRANSFORMER_14791867367496.

The reference decoder is affine in the positions: each frame step is
    pos_{t+1} = pos_t @ M + (d_t[b] + g[b,j]),   M = I + W_pe @ W3  (3x3)
(with W_final = [W1; W2; W3] split along its 768 input rows), so the whole
60-step scan has a closed form

    out[b, j, :, t] = X[b, j, :] @ Q_t + r_t[b, :]

where X = initial_grid,
    Q_t = M^t + (W_pe @ W2) @ S_t,          S_t = sum_{k<t} M^k
    r_t[b] = h @ S_t + D_t[b],              D_t = sum_{s=1..t} d_s M^{t-s}
    d_t[b] = (emb_table[t] + z @ W_clip + b_clip) @ W1
    h      = b_pe @ (W2 + W3) + b_final

All of Q/r are tiny (3x3 / per-batch 3-vectors) and are computed on the host
in float64.  The device kernel is then a single affine map per point
([3 feats + bias] -> 180 outputs).

Precision budget: the harness gate is rel_err < 2e-2, so inputs are plain
bf16 (X and Q single-chunk; the large accumulated offset r split into two
bf16 chunks against two 1.0 bias rows) and the output is written as bf16
then upcast to f32 on gather.  Measured end-to-end error ~1.7e-3.  K-stack
per tile is 5 rows, two tiles fused block-diagonally per matmul (K=10,
N=360).

Machine model (all measured via NTFF traces):
 - PE issues a [10,128]x[10,360] matmul every ~253-300 ns (psum write port);
   64 matmuls are the steady-state floor.
 - DVE/ACT psum->sbuf copy cost is per-instruction-bubble + free-dim:
   DVE ~ (120 + FD)/0.96GHz, ACT ~ (172 + FD)/1.2GHz.  Copying TWO psum
   slots per instruction (FD=720, 2D access pattern over two bank-aligned
   slots) amortizes the bubble so the two copy engines together drain
   faster than PE fills -- the kernel is then PE-bound, not copy-bound.
 - Input DRAM->SBUF DMA: each partition row moves as serial ~1KB packets at
   ~350ns each, rows spread across engines in parallel; queue spin-up after
   issue is ~1.7us.  So the first matmul's gating DMA carries <=1KB/row
   (rhs for batch-pair 0 + stationary slab 0 = 976B/row) and later data
   trickles in behind on other rings.  Warm-up matmuls on garbage keep PE
   busy so the real wait never enters event-sleep.
 - The walrus NEFF epilogue resets the whole 256-entry semaphore file,
   ~51 resets per engine at 45-115ns each (~6us on the Tensor queue).
   --max-sem-num shrinks the file and with it the epilogue.

Sharding: data-parallel over batch -- each of the 8 cores handles 4
batches (16384 points = 128 point-tiles = 64 packed matmuls).
"""

import numpy as np

BS, NFRAMES, NJOINTS, NFEATS, LATENT, CLIP = 32, 60, 4096, 3, 256, 512
NCORES = 8
B_PER_CORE = BS // NCORES                  # 4
PTS = B_PER_CORE * NJOINTS                 # 16384 points per core
NTILES = PTS // 128                        # 128 point-tiles per core
GROUPS = 8                                 # output DMA groups
TPG = NTILES // GROUPS                     # 16 tiles per group
FC = NFEATS * NFRAMES                      # 180 output columns per point
KR = 4                                     # K rows per tile (3 feats + 1 bias)
PAIR = 2                                   # tiles fused per matmul
MM_PER_G = TPG // PAIR                     # 8 matmuls per group
N_WARM = 14                                # PE warm-up matmuls
N_JUNK = 8                                 # PE post-work filler matmuls


def _split2(a):
    """Split f32 array into two bf16 chunks whose sum reproduces ~16
    mantissa bits.  Returned as f32 arrays holding bf16-representable
    values."""
    import ml_dtypes
    bf = ml_dtypes.bfloat16
    a = np.asarray(a, np.float32)
    a0 = a.astype(bf).astype(np.float32)
    a1 = (a - a0).astype(bf).astype(np.float32)
    return a0, a1


def _precompute(z, W_pe, b_pe, W_clip, b_clip, emb_table, W_final, b_final):
    """Host-side f64 computation of the closed-form coefficients.

    Returns Q_all [3, 180] and r_all [32, 180], column layout c = f*60 + t
    (matching the [.., 3, 60] innermost layout of the output)."""
    f64 = np.float64
    W_pe64 = np.asarray(W_pe, f64)
    W_fin = np.asarray(W_final, f64)
    W1, W2, W3 = W_fin[:LATENT], W_fin[LATENT:2 * LATENT], W_fin[2 * LATENT:]
    M = np.eye(3) + W_pe64 @ W3
    Gm = W_pe64 @ W2
    b_pe64 = np.asarray(b_pe, f64)
    h = b_pe64 @ W2 + b_pe64 @ W3 + np.asarray(b_final, f64)
    z_proj = np.asarray(z, f64) @ np.asarray(W_clip, f64) + np.asarray(b_clip, f64)
    d = (np.asarray(emb_table, f64)[None, :, :] + z_proj[:, None, :]) @ W1  # [32,60,3]

    Q = np.zeros((NFRAMES, 3, 3))
    R = np.zeros((NFRAMES, BS, 3))
    Q[0] = np.eye(3)
    Mt = np.eye(3)
    S = np.zeros((3, 3))
    D = np.zeros((BS, 3))
    for t in range(1, NFRAMES):
        S = S + Mt
        Mt = Mt @ M
        D = D @ M + d[:, t, :]
        Q[t] = Mt + Gm @ S
        R[t] = h @ S + D
    Q_all = Q.transpose(1, 2, 0).reshape(3, FC)     # [k, f*60+t]
    r_all = R.transpose(1, 2, 0).reshape(BS, FC)    # [b, f*60+t]
    return Q_all.astype(np.float32), r_all.astype(np.float32)


N_STAGE = 3    # stage buffers
PW = 512       # psum pair-slot half stride (one 2KB bank, in f32 elems)
NP = 4         # psum pair tensors (= 8 matmul slots)


def _build_bass():
    import concourse.mybir as mybir
    from concourse import bacc
    from concourse.bass import ts

    f32 = mybir.dt.float32
    bf16 = mybir.dt.bfloat16
    nc = bacc.Bacc(None, target_bir_lowering=False)
    # inputs: in00 (2 packets per partition row) gates matmuls 0-3, in01
    # (1 packet) gates matmuls 4-7, in02 gates matmul 16 (rhs for
    # batch-pairs 1-3), chunks 1-7 gate their groups.
    in00 = nc.dram_tensor("in00", [PAIR * KR, FC * PAIR + 4 * 128],
                          bf16, kind="ExternalInput")  # rhs lb0 + slabs 0-3
    in01 = nc.dram_tensor("in01", [PAIR * KR, 4 * 128], bf16,
                          kind="ExternalInput")       # slabs 4-7
    in02 = nc.dram_tensor("in02", [PAIR * KR, 3 * FC * PAIR], bf16,
                          kind="ExternalInput")       # rhs lb 1-3
    xt = nc.dram_tensor("xt", [PAIR * KR, NTILES // PAIR * 128], bf16,
                        kind="ExternalInput")         # cols [1024g, 1024(g+1)) = group g
    out = nc.dram_tensor("out", [PTS, FC], bf16, kind="ExternalOutput")
    out_v = out[:].rearrange("(g j w) c -> g j (w c)", g=GROUPS, j=128, w=TPG)
    out_v4 = out[:].rearrange("(g j w) c -> g j w c", g=GROUPS, j=128, w=TPG)

    from contextlib import ExitStack
    ctx = ExitStack()
    in00_sb = ctx.enter_context(
        nc.sbuf_tensor("in00_sb", [PAIR * KR, FC * PAIR + 4 * 128], bf16))
    in01_sb = ctx.enter_context(
        nc.sbuf_tensor("in01_sb", [PAIR * KR, 4 * 128], bf16))
    in02_sb = ctx.enter_context(
        nc.sbuf_tensor("in02_sb", [PAIR * KR, 3 * FC * PAIR], bf16))
    xt_sb = [None] + [ctx.enter_context(
        nc.sbuf_tensor(f"xt_sb{g}", [PAIR * KR, MM_PER_G * 128], bf16))
        for g in range(1, GROUPS)]
    stage = [ctx.enter_context(
        nc.sbuf_tensor(f"stage{i}", [128, TPG * FC], bf16))
        for i in range(N_STAGE)]
    # 4 psum tensors of two 2KB banks each; matmul slot m (0-7) is tensor
    # m//2, column half m%2.  One copy drains a whole tensor (both halves)
    # with a 2D access pattern.
    psum = [ctx.enter_context(
        nc.psum_tensor(f"psum{i}", [128, 2 * PW], f32))
        for i in range(NP)]
    s_c00 = ctx.enter_context(nc.semaphore("s_c00"))
    s_c01 = ctx.enter_context(nc.semaphore("s_c01"))
    s_c02 = ctx.enter_context(nc.semaphore("s_c02"))
    s_chunk = [None] + [ctx.enter_context(nc.semaphore(f"s_chunk{g}"))
                        for g in range(1, GROUPS)]
    s_pe = ctx.enter_context(nc.semaphore("s_pe"))
    s_cpv = ctx.enter_context(nc.semaphore("s_cpv"))
    s_cpa = ctx.enter_context(nc.semaphore("s_cpa"))
    s_slot = [ctx.enter_context(nc.semaphore(f"s_slot{i}"))
              for i in range(N_STAGE)]

    # ---- input DMAs.  Each dma_start's partition rows move as serial
    # ~1KB packets spread across DMA engines, so the gating DMA is one
    # packet per row and the rest trickle in on parallel rings. ----
    nc.sync.dma_start(out=in00_sb[:], in_=in00[:]).then_inc(s_c00, 16)
    nc.sync.dma_start(out=in01_sb[:], in_=in01[:]).then_inc(s_c01, 16)
    nc.sync.dma_start(out=in02_sb[:], in_=in02[:]).then_inc(s_c02, 16)
    ring = {1: nc.gpsimd, 2: nc.scalar, 3: nc.scalar, 4: nc.gpsimd,
            5: nc.gpsimd, 6: nc.gpsimd, 7: nc.sync}
    for g in range(1, GROUPS):
        ring[g].dma_start(
            out=xt_sb[g][:], in_=xt[:, ts(g, MM_PER_G * 128)]
        ).then_inc(s_chunk[g], 16)

    def stat_ap(g, sp):
        """Stationary [K, 128] slab for matmul sp of group g."""
        if g == 0:
            if sp < 4:
                return in00_sb[:, FC * PAIR + 128 * sp:
                               FC * PAIR + 128 * (sp + 1)]
            return in01_sb[:, ts(sp - 4, 128)]
        return xt_sb[g][:, ts(sp, 128)]

    def rhs_ap(lb):
        """Moving [K, 360] operand for batch-pair lb."""
        if lb == 0:
            return in00_sb[:, :FC * PAIR]
        return in02_sb[:, ts(lb - 1, FC * PAIR)]

    def psum_mm(j):
        """Matmul j's psum destination (bank-aligned half of a pair)."""
        m = j % (2 * NP)
        return psum[m // 2][:, (m % 2) * PW:(m % 2) * PW + FC * PAIR]

    def psum_pair_in(c):
        """Copy source covering matmul pair (2c, 2c+1): 2D AP over both
        bank-aligned halves of one psum tensor."""
        return psum[c % NP][:].rearrange(
            "p (s w) -> p s w", s=2)[:, :, :FC * PAIR]

    # copy schedule: groups 0-6 use paired copies c = 0..27 alternating
    # DVE (even c) / ACT (odd c); group 7 uses single copies s = 0..7
    # alternating DVE (even s) / ACT (odd s) so the tail pieces are small.
    N_PC = 4 * (GROUPS - 1)                 # 28 paired copies

    def copy_engine_pos(c):
        """(engine, 1-based position in that engine's stream) of paired
        copy c."""
        return ("v" if c % 2 == 0 else "a"), c // 2 + 1

    V_PAIRS = N_PC // 2                     # 14 per engine
    # out-DMA piece counts per group (for stage-slot reuse bookkeeping)
    dma_count = {0: 4, GROUPS - 2: 2, GROUPS - 1: 3}
    slot_reads_before = {}
    seen = [0] * N_STAGE
    for g in range(GROUPS):
        slot_reads_before[g] = seen[g % N_STAGE]
        seen[g % N_STAGE] += dma_count.get(g, 1)

    def stage_pair_out(g, l):
        """Stage destination of paired copy l (0-3) of group g, as a 2D
        view matching psum_pair_in."""
        return stage[g % N_STAGE][:].rearrange(
            "p (l s w) -> p l s w", l=4, s=2)[:, l, :, :]

    def emit_copies(engine, sem, parity):
        """parity 0 = DVE stream, 1 = ACT stream."""
        for c in range(parity, N_PC, 2):
            g, l = c // 4, c % 4
            if l == parity and g >= N_STAGE:
                engine.wait_ge(s_slot[g % N_STAGE],
                               16 * slot_reads_before[g])
            engine.wait_ge(s_pe, 2 * c + 2)
            if parity == 0:
                nc.vector.tensor_copy(
                    out=stage_pair_out(g, l), in_=psum_pair_in(c)
                ).then_inc(sem, 1)
            else:
                nc.scalar.copy(
                    out=stage_pair_out(g, l), in_=psum_pair_in(c)
                ).then_inc(sem, 1)
        # group 7 singles
        g = GROUPS - 1
        st = stage[g % N_STAGE]
        for s in range(parity, MM_PER_G, 2):
            j = (GROUPS - 1) * MM_PER_G + s
            if s == parity:
                engine.wait_ge(s_slot[g % N_STAGE],
                               16 * slot_reads_before[g])
            engine.wait_ge(s_pe, j + 1)
            if parity == 0:
                nc.vector.tensor_copy(
                    out=st[:, ts(s, PAIR * FC)], in_=psum_mm(j)
                ).then_inc(sem, 1)
            else:
                nc.scalar.copy(
                    out=st[:, ts(s, PAIR * FC)], in_=psum_mm(j)
                ).then_inc(sem, 1)

    emit_copies(nc.scalar, s_cpa, 1)
    # last 2-tile piece of the last group: issued on the ACT ring right
    # after ACT's own final single copy (engine stream order makes a
    # semaphore wait unnecessary), so it does not queue behind SP's
    # earlier piece issues.
    g_last = GROUPS - 1
    nc.scalar.dma_start(
        out=out_v4[g_last][:, 14:TPG, :],
        in_=stage[g_last % N_STAGE][:, 14 * FC:TPG * FC],
    ).then_inc(s_slot[g_last % N_STAGE], 16)
    emit_copies(nc.vector, s_cpv, 0)
    # 6-tile piece of the last group on the GPSIMD ring (idle by then,
    # so it issues in parallel with SP's and ACT's pieces)
    nc.gpsimd.wait_ge(s_cpv, V_PAIRS + 4)
    nc.gpsimd.wait_ge(s_cpa, V_PAIRS + 3)
    nc.gpsimd.dma_start(
        out=out_v4[g_last][:, 8:14, :],
        in_=stage[g_last % N_STAGE][:, 8 * FC:14 * FC],
    ).then_inc(s_slot[g_last % N_STAGE], 16)

    # ---- PE: warm-up matmuls on garbage data (keep the engine busy until
    # the gating DMA's semaphore is already set, avoiding the event-sleep
    # wake penalty), then the real matmuls ----
    for w in range(N_WARM):
        nc.tensor.matmul(
            psum_mm(2 * NP - 1),
            xt_sb[GROUPS - 1][:, ts(w % MM_PER_G, 128)],
            in00_sb[:, :PAIR * FC],
            start=True, stop=True,
        )
    for g in range(GROUPS):
        lb = g // 2
        for sp in range(MM_PER_G):
            j = g * MM_PER_G + sp
            if g == 0:
                if sp == 0:
                    nc.tensor.wait_ge(s_c00, 16)
                elif sp == 4:
                    nc.tensor.wait_ge(s_c01, 16)
            elif sp == 0:
                nc.tensor.wait_ge(s_chunk[g], 16)
                if g == 2:
                    nc.tensor.wait_ge(s_c02, 16)
            if j >= 2 * NP and j % 2 == 0:
                # psum pair-slot reuse: wait for the paired copy that
                # drained it (or, for the last group, handled below)
                cc = (j - 2 * NP) // 2
                eng, pos = copy_engine_pos(cc)
                nc.tensor.wait_ge(s_cpv if eng == "v" else s_cpa, pos)
            nc.tensor.matmul(
                psum_mm(j),
                stat_ap(g, sp),
                rhs_ap(lb),
                start=True, stop=True,
            ).then_inc(s_pe, 1)
    # post-work filler matmuls (no semaphore update, garbage psum slot 0
    # whose copies finished long ago): keep PE busy until the other
    # engines reach the end-of-kernel barrier so PE's barrier wait does
    # not enter event-sleep (~0.6us wake penalty before the runtime's
    # semaphore-reset epilogue, which runs on PE's queue).
    for w in range(N_JUNK):
        nc.tensor.matmul(
            psum_mm(0),
            xt_sb[GROUPS - 1][:, ts(w % MM_PER_G, 128)],
            in00_sb[:, :PAIR * FC],
            start=True, stop=True,
        )

    # ---- SP: output DMAs ----
    for g in range(GROUPS):
        st = stage[g % N_STAGE]
        if g == 0:
            # four quarter-DMAs, each gated on a single paired copy, so
            # the output stream starts right after the first copy lands
            for nv, na, w0, w1 in ((1, 0, 0, 4), (0, 1, 4, 8),
                                   (2, 0, 8, 12), (0, 2, 12, TPG)):
                if nv:
                    nc.sync.wait_ge(s_cpv, nv)
                if na:
                    nc.sync.wait_ge(s_cpa, na)
                nc.sync.dma_start(
                    out=out_v4[0][:, w0:w1, :],
                    in_=st[:, w0 * FC:w1 * FC],
                ).then_inc(s_slot[0], 16)
            continue
        if g == GROUPS - 2:
            # second-to-last group in halves to start its drain earlier
            for q in range(2):
                w0, w1 = q * TPG // 2, (q + 1) * TPG // 2
                n = 2 * g + 1 + q
                nc.sync.wait_ge(s_cpv, n)
                nc.sync.wait_ge(s_cpa, n)
                nc.sync.dma_start(
                    out=out_v4[g][:, w0:w1, :],
                    in_=st[:, w0 * FC:w1 * FC],
                ).then_inc(s_slot[g % N_STAGE], 16)
            continue
        if g == GROUPS - 1:
            # first 8-tile piece of the last group (singles s0-s3); the
            # 6- and 2-tile pieces are issued on the GPSIMD and ACT rings
            # (emitted above) so the three issues overlap
            nc.sync.wait_ge(s_cpv, V_PAIRS + 2)
            nc.sync.wait_ge(s_cpa, V_PAIRS + 2)
            nc.sync.dma_start(
                out=out_v4[g][:, 0:8, :],
                in_=st[:, 0:8 * FC],
            ).then_inc(s_slot[g % N_STAGE], 16)
            continue
        nc.sync.wait_ge(s_cpv, 2 * (g + 1))
        nc.sync.wait_ge(s_cpa, 2 * (g + 1))
        nc.sync.dma_start(out=out_v[g], in_=st[:]).then_inc(
            s_slot[g % N_STAGE], 16)

    ctx.close()
    nc.finalize()
    return nc


_NC_CACHE = None
_LAST_RESULTS = None  # BassKernelResults of the most recent run (for profiling)


def kernel(z, mask, initial_grid, W_pe, b_pe, W_clip, b_clip, emb_table,
           W_final, b_final):
    global _NC_CACHE, _LAST_RESULTS
    import ml_dtypes
    from concourse import bass_utils

    bf = ml_dtypes.bfloat16
    Q_all, r_all = _precompute(z, W_pe, b_pe, W_clip, b_clip, emb_table,
                               W_final, b_final)
    Q0 = Q_all.astype(bf).astype(np.float32)            # [3, 180]
    X = np.ascontiguousarray(np.asarray(initial_grid), dtype=np.float32)

    in_maps = []
    for c in range(NCORES):
        Xc = X[B_PER_CORE * c:B_PER_CORE * (c + 1)].reshape(PTS, NFEATS)
        # point p = g*2048 + j*16 + w lives at tile (g, w), psum partition j
        X4 = Xc.reshape(GROUPS, 128, TPG, NFEATS).transpose(3, 0, 2, 1)
        A = np.empty((GROUPS, TPG, KR, 128), np.float32)
        A[:, :, 0:NFEATS, :] = X4.transpose(1, 2, 0, 3)  # single bf16 chunk
        A[:, :, NFEATS:KR, :] = 1.0                      # bias row (r)
        # matmul s covers tiles (2*(s%8), 2*(s%8)+1) of group s//8;
        # stationary rows KR*a.. hold tile a of the pair
        xt_host = (A.reshape(GROUPS, MM_PER_G, PAIR, KR, 128)
                   .transpose(2, 3, 0, 1, 4)
                   .reshape(PAIR * KR, NTILES // PAIR * 128)).astype(bf)

        rhs_host = np.zeros((PAIR * KR, B_PER_CORE * PAIR * FC), np.float32)
        for lb in range(B_PER_CORE):
            R = np.empty((KR, FC), np.float32)
            R[0:NFEATS] = Q0
            R[NFEATS] = r_all[B_PER_CORE * c + lb]
            for a in range(PAIR):                       # block-diagonal
                rhs_host[KR * a:KR * (a + 1),
                         lb * PAIR * FC + FC * a: lb * PAIR * FC + FC * (a + 1)] = R
        rhs_host = rhs_host.astype(bf)
        in00_host = np.concatenate(
            [rhs_host[:, :PAIR * FC], xt_host[:, :4 * 128]], axis=1)
        in01_host = xt_host[:, 4 * 128:MM_PER_G * 128]
        in02_host = rhs_host[:, PAIR * FC:]
        in_maps.append({"xt": np.ascontiguousarray(xt_host),
                        "in00": np.ascontiguousarray(in00_host),
                        "in01": np.ascontiguousarray(in01_host),
                        "in02": np.ascontiguousarray(in02_host)})

    if _NC_CACHE is None:
        _NC_CACHE = _build_bass()
    res = bass_utils.run_bass_kernel_spmd(
        _NC_CACHE, in_maps, core_ids=list(range(NCORES))
    )
    _LAST_RESULTS = res

    out = np.empty((BS, NJOINTS, NFEATS, NFRAMES), np.float32)
    for c in range(NCORES):
        out[B_PER_CORE * c:B_PER_CORE * (c + 1)] = (
            np.asarray(res.results[c]["out"]).astype(np.float32)
            .reshape(B_PER_CORE, NJOINTS, NFEATS, NFRAMES)
        )
    return out


# revision 14
# speedup vs baseline: 1.2931x; 1.2931x over previous
"""Trainium2 Bass kernel for nn_Decoder_TRANSFORMER_14791867367496.

The reference decoder is affine in the positions: each frame step is
    pos_{t+1} = pos_t @ M + (d_t[b] + g[b,j]),   M = I + W_pe @ W3  (3x3)
(with W_final = [W1; W2; W3] split along its 768 input rows), so the whole
60-step scan has a closed form

    out[b, j, :, t] = X[b, j, :] @ Q_t + r_t[b, :]

where X = initial_grid,
    Q_t = M^t + (W_pe @ W2) @ S_t,          S_t = sum_{k<t} M^k
    r_t[b] = h @ S_t + D_t[b],              D_t = sum_{s=1..t} d_s M^{t-s}
    d_t[b] = (emb_table[t] + z @ W_clip + b_clip) @ W1
    h      = b_pe @ (W2 + W3) + b_final

All of Q/r are tiny (3x3 / per-batch 3-vectors) and are computed on the host
in float64.  The device kernel is then a single affine map per point
([3 feats + bias] -> 180 outputs).

Precision budget: the harness gate is rel_err < 2e-2, so inputs are plain
bf16 (X and Q single-chunk; the large accumulated offset r split into two
bf16 chunks against two 1.0 bias rows) and the output is written as bf16
then upcast to f32 on gather.  Measured end-to-end error ~1.7e-3.  K-stack
per tile is 5 rows, two tiles fused block-diagonally per matmul (K=10,
N=360).

Machine model (all measured via NTFF traces):
 - PE issues a [10,128]x[10,360] matmul every ~253-300 ns (psum write port);
   64 matmuls are the steady-state floor.
 - DVE/ACT psum->sbuf copy cost is per-instruction-bubble + free-dim:
   DVE ~ (120 + FD)/0.96GHz, ACT ~ (172 + FD)/1.2GHz.  Copying TWO psum
   slots per instruction (FD=720, 2D access pattern over two bank-aligned
   slots) amortizes the bubble so the two copy engines together drain
   faster than PE fills -- the kernel is then PE-bound, not copy-bound.
 - Input DRAM->SBUF DMA: each partition row moves as serial ~1KB packets at
   ~350ns each, rows spread across engines in parallel; queue spin-up after
   issue is ~1.7us.  So the first matmul's gating DMA carries <=1KB/row
   (rhs for batch-pair 0 + stationary slab 0 = 976B/row) and later data
   trickles in behind on other rings.  Warm-up matmuls on garbage keep PE
   busy so the real wait never enters event-sleep.
 - The walrus NEFF epilogue resets the whole 256-entry semaphore file,
   ~51 resets per engine at 45-115ns each (~6us on the Tensor queue).
   --max-sem-num shrinks the file and with it the epilogue.

Sharding: data-parallel over batch -- each of the 8 cores handles 4
batches (16384 points = 128 point-tiles = 64 packed matmuls).
"""

import numpy as np

BS, NFRAMES, NJOINTS, NFEATS, LATENT, CLIP = 32, 60, 4096, 3, 256, 512
NCORES = 8
B_PER_CORE = BS // NCORES                  # 4
PTS = B_PER_CORE * NJOINTS                 # 16384 points per core
NTILES = PTS // 128                        # 128 point-tiles per core
GROUPS = 8                                 # output DMA groups
TPG = NTILES // GROUPS                     # 16 tiles per group
FC = NFEATS * NFRAMES                      # 180 output columns per point
KR = 5                                     # K rows per tile (3 feats + 2 bias)
PAIR = 2                                   # tiles fused per matmul
MM_PER_G = TPG // PAIR                     # 8 matmuls per group
N_WARM = 9                                 # PE warm-up matmuls
N_JUNK = 4                                 # PE post-work filler matmuls


def _split2(a):
    """Split f32 array into two bf16 chunks whose sum reproduces ~16
    mantissa bits.  Returned as f32 arrays holding bf16-representable
    values."""
    import ml_dtypes
    bf = ml_dtypes.bfloat16
    a = np.asarray(a, np.float32)
    a0 = a.astype(bf).astype(np.float32)
    a1 = (a - a0).astype(bf).astype(np.float32)
    return a0, a1


def _precompute(z, W_pe, b_pe, W_clip, b_clip, emb_table, W_final, b_final):
    """Host-side f64 computation of the closed-form coefficients.

    Returns Q_all [3, 180] and r_all [32, 180], column layout c = f*60 + t
    (matching the [.., 3, 60] innermost layout of the output)."""
    f64 = np.float64
    W_pe64 = np.asarray(W_pe, f64)
    W_fin = np.asarray(W_final, f64)
    W1, W2, W3 = W_fin[:LATENT], W_fin[LATENT:2 * LATENT], W_fin[2 * LATENT:]
    M = np.eye(3) + W_pe64 @ W3
    Gm = W_pe64 @ W2
    b_pe64 = np.asarray(b_pe, f64)
    h = b_pe64 @ W2 + b_pe64 @ W3 + np.asarray(b_final, f64)
    z_proj = np.asarray(z, f64) @ np.asarray(W_clip, f64) + np.asarray(b_clip, f64)
    d = (np.asarray(emb_table, f64)[None, :, :] + z_proj[:, None, :]) @ W1  # [32,60,3]

    Q = np.zeros((NFRAMES, 3, 3))
    R = np.zeros((NFRAMES, BS, 3))
    Q[0] = np.eye(3)
    Mt = np.eye(3)
    S = np.zeros((3, 3))
    D = np.zeros((BS, 3))
    for t in range(1, NFRAMES):
        S = S + Mt
        Mt = Mt @ M
        D = D @ M + d[:, t, :]
        Q[t] = Mt + Gm @ S
        R[t] = h @ S + D
    Q_all = Q.transpose(1, 2, 0).reshape(3, FC)     # [k, f*60+t]
    r_all = R.transpose(1, 2, 0).reshape(BS, FC)    # [b, f*60+t]
    return Q_all.astype(np.float32), r_all.astype(np.float32)


N_STAGE = 3    # stage buffers
PW = 512       # psum pair-slot half stride (one 2KB bank, in f32 elems)
NP = 4         # psum pair tensors (= 8 matmul slots)


def _build_bass():
    import concourse.mybir as mybir
    from concourse import bacc
    from concourse.bass import ts

    f32 = mybir.dt.float32
    bf16 = mybir.dt.bfloat16
    nc = bacc.Bacc(None, target_bir_lowering=False)
    # inputs: in00 (2 packets per partition row) gates matmuls 0-3, in01
    # (1 packet) gates matmuls 4-7, in02 gates matmul 16 (rhs for
    # batch-pairs 1-3), chunks 1-7 gate their groups.
    in00 = nc.dram_tensor("in00", [PAIR * KR, FC * PAIR + 4 * 128],
                          bf16, kind="ExternalInput")  # rhs lb0 + slabs 0-3
    in01 = nc.dram_tensor("in01", [PAIR * KR, 4 * 128], bf16,
                          kind="ExternalInput")       # slabs 4-7
    in02 = nc.dram_tensor("in02", [PAIR * KR, 3 * FC * PAIR], bf16,
                          kind="ExternalInput")       # rhs lb 1-3
    xt = nc.dram_tensor("xt", [PAIR * KR, NTILES // PAIR * 128], bf16,
                        kind="ExternalInput")         # cols [1024g, 1024(g+1)) = group g
    out = nc.dram_tensor("out", [PTS, FC], bf16, kind="ExternalOutput")
    out_v = out[:].rearrange("(g j w) c -> g j (w c)", g=GROUPS, j=128, w=TPG)
    out_v4 = out[:].rearrange("(g j w) c -> g j w c", g=GROUPS, j=128, w=TPG)

    from contextlib import ExitStack
    ctx = ExitStack()
    in00_sb = ctx.enter_context(
        nc.sbuf_tensor("in00_sb", [PAIR * KR, FC * PAIR + 4 * 128], bf16))
    in01_sb = ctx.enter_context(
        nc.sbuf_tensor("in01_sb", [PAIR * KR, 4 * 128], bf16))
    in02_sb = ctx.enter_context(
        nc.sbuf_tensor("in02_sb", [PAIR * KR, 3 * FC * PAIR], bf16))
    xt_sb = [None] + [ctx.enter_context(
        nc.sbuf_tensor(f"xt_sb{g}", [PAIR * KR, MM_PER_G * 128], bf16))
        for g in range(1, GROUPS)]
    stage = [ctx.enter_context(
        nc.sbuf_tensor(f"stage{i}", [128, TPG * FC], bf16))
        for i in range(N_STAGE)]
    # 4 psum tensors of two 2KB banks each; matmul slot m (0-7) is tensor
    # m//2, column half m%2.  One copy drains a whole tensor (both halves)
    # with a 2D access pattern.
    psum = [ctx.enter_context(
        nc.psum_tensor(f"psum{i}", [128, 2 * PW], f32))
        for i in range(NP)]
    s_c00 = ctx.enter_context(nc.semaphore("s_c00"))
    s_c01 = ctx.enter_context(nc.semaphore("s_c01"))
    s_c02 = ctx.enter_context(nc.semaphore("s_c02"))
    s_chunk = [None] + [ctx.enter_context(nc.semaphore(f"s_chunk{g}"))
                        for g in range(1, GROUPS)]
    s_pe = ctx.enter_context(nc.semaphore("s_pe"))
    s_cpv = ctx.enter_context(nc.semaphore("s_cpv"))
    s_cpa = ctx.enter_context(nc.semaphore("s_cpa"))
    s_slot = [ctx.enter_context(nc.semaphore(f"s_slot{i}"))
              for i in range(N_STAGE)]

    # ---- input DMAs.  Each dma_start's partition rows move as serial
    # ~1KB packets spread across DMA engines, so the gating DMA is one
    # packet per row and the rest trickle in on parallel rings. ----
    nc.sync.dma_start(out=in00_sb[:], in_=in00[:]).then_inc(s_c00, 16)
    nc.sync.dma_start(out=in01_sb[:], in_=in01[:]).then_inc(s_c01, 16)
    nc.sync.dma_start(out=in02_sb[:], in_=in02[:]).then_inc(s_c02, 16)
    ring = {1: nc.gpsimd, 2: nc.scalar, 3: nc.scalar, 4: nc.gpsimd,
            5: nc.gpsimd, 6: nc.gpsimd, 7: nc.sync}
    for g in range(1, GROUPS):
        ring[g].dma_start(
            out=xt_sb[g][:], in_=xt[:, ts(g, MM_PER_G * 128)]
        ).then_inc(s_chunk[g], 16)

    def stat_ap(g, sp):
        """Stationary [K, 128] slab for matmul sp of group g."""
        if g == 0:
            if sp < 4:
                return in00_sb[:, FC * PAIR + 128 * sp:
                               FC * PAIR + 128 * (sp + 1)]
            return in01_sb[:, ts(sp - 4, 128)]
        return xt_sb[g][:, ts(sp, 128)]

    def rhs_ap(lb):
        """Moving [K, 360] operand for batch-pair lb."""
        if lb == 0:
            return in00_sb[:, :FC * PAIR]
        return in02_sb[:, ts(lb - 1, FC * PAIR)]

    def psum_mm(j):
        """Matmul j's psum destination (bank-aligned half of a pair)."""
        m = j % (2 * NP)
        return psum[m // 2][:, (m % 2) * PW:(m % 2) * PW + FC * PAIR]

    def psum_pair_in(c):
        """Copy source covering matmul pair (2c, 2c+1): 2D AP over both
        bank-aligned halves of one psum tensor."""
        return psum[c % NP][:].rearrange(
            "p (s w) -> p s w", s=2)[:, :, :FC * PAIR]

    # copy schedule: groups 0-6 use paired copies c = 0..27 alternating
    # DVE (even c) / ACT (odd c); group 7 uses single copies s = 0..7
    # alternating DVE (even s) / ACT (odd s) so the tail pieces are small.
    N_PC = 4 * (GROUPS - 1)                 # 28 paired copies

    def copy_engine_pos(c):
        """(engine, 1-based position in that engine's stream) of paired
        copy c."""
        return ("v" if c % 2 == 0 else "a"), c // 2 + 1

    V_PAIRS = N_PC // 2                     # 14 per engine
    # out-DMA piece counts per group (for stage-slot reuse bookkeeping)
    dma_count = {0: 4, GROUPS - 2: 2, GROUPS - 1: 3}
    slot_reads_before = {}
    seen = [0] * N_STAGE
    for g in range(GROUPS):
        slot_reads_before[g] = seen[g % N_STAGE]
        seen[g % N_STAGE] += dma_count.get(g, 1)

    def stage_pair_out(g, l):
        """Stage destination of paired copy l (0-3) of group g, as a 2D
        view matching psum_pair_in."""
        return stage[g % N_STAGE][:].rearrange(
            "p (l s w) -> p l s w", l=4, s=2)[:, l, :, :]

    def emit_copies(engine, sem, parity):
        """parity 0 = DVE stream, 1 = ACT stream."""
        for c in range(parity, N_PC, 2):
            g, l = c // 4, c % 4
            if l == parity and g >= N_STAGE:
                engine.wait_ge(s_slot[g % N_STAGE],
                               16 * slot_reads_before[g])
            engine.wait_ge(s_pe, 2 * c + 2)
            if parity == 0:
                nc.vector.tensor_copy(
                    out=stage_pair_out(g, l), in_=psum_pair_in(c)
                ).then_inc(sem, 1)
            else:
                nc.scalar.copy(
                    out=stage_pair_out(g, l), in_=psum_pair_in(c)
                ).then_inc(sem, 1)
        # group 7 singles
        g = GROUPS - 1
        st = stage[g % N_STAGE]
        for s in range(parity, MM_PER_G, 2):
            j = (GROUPS - 1) * MM_PER_G + s
            if s == parity:
                engine.wait_ge(s_slot[g % N_STAGE],
                               16 * slot_reads_before[g])
            engine.wait_ge(s_pe, j + 1)
            if parity == 0:
                nc.vector.tensor_copy(
                    out=st[:, ts(s, PAIR * FC)], in_=psum_mm(j)
                ).then_inc(sem, 1)
            else:
                nc.scalar.copy(
                    out=st[:, ts(s, PAIR * FC)], in_=psum_mm(j)
                ).then_inc(sem, 1)

    emit_copies(nc.scalar, s_cpa, 1)
    # last 2-tile piece of the last group: issued on the ACT ring right
    # after ACT's own final single copy (engine stream order makes a
    # semaphore wait unnecessary), so it does not queue behind SP's
    # earlier piece issues.
    g_last = GROUPS - 1
    nc.scalar.dma_start(
        out=out_v4[g_last][:, 14:TPG, :],
        in_=stage[g_last % N_STAGE][:, 14 * FC:TPG * FC],
    ).then_inc(s_slot[g_last % N_STAGE], 16)
    emit_copies(nc.vector, s_cpv, 0)
    # 6-tile piece of the last group on the GPSIMD ring (idle by then,
    # so it issues in parallel with SP's and ACT's pieces)
    nc.gpsimd.wait_ge(s_cpv, V_PAIRS + 4)
    nc.gpsimd.wait_ge(s_cpa, V_PAIRS + 3)
    nc.gpsimd.dma_start(
        out=out_v4[g_last][:, 8:14, :],
        in_=stage[g_last % N_STAGE][:, 8 * FC:14 * FC],
    ).then_inc(s_slot[g_last % N_STAGE], 16)

    # ---- PE: warm-up matmuls on garbage data (keep the engine busy until
    # the gating DMA's semaphore is already set, avoiding the event-sleep
    # wake penalty), then the real matmuls ----
    for w in range(N_WARM):
        nc.tensor.matmul(
            psum_mm(2 * NP - 1),
            xt_sb[GROUPS - 1][:, ts(w % MM_PER_G, 128)],
            in00_sb[:, :PAIR * FC],
            start=True, stop=True,
        )
    for g in range(GROUPS):
        lb = g // 2
        for sp in range(MM_PER_G):
            j = g * MM_PER_G + sp
            if g == 0:
                if sp == 0:
                    nc.tensor.wait_ge(s_c00, 16)
                elif sp == 4:
                    nc.tensor.wait_ge(s_c01, 16)
            elif sp == 0:
                nc.tensor.wait_ge(s_chunk[g], 16)
                if g == 2:
                    nc.tensor.wait_ge(s_c02, 16)
            if j >= 2 * NP and j % 2 == 0:
                # psum pair-slot reuse: wait for the paired copy that
                # drained it (or, for the last group, handled below)
                cc = (j - 2 * NP) // 2
                eng, pos = copy_engine_pos(cc)
                nc.tensor.wait_ge(s_cpv if eng == "v" else s_cpa, pos)
            nc.tensor.matmul(
                psum_mm(j),
                stat_ap(g, sp),
                rhs_ap(lb),
                start=True, stop=True,
            ).then_inc(s_pe, 1)
    # post-work filler matmuls (no semaphore update, garbage psum slot 0
    # whose copies finished long ago): keep PE busy until the other
    # engines reach the end-of-kernel barrier so PE's barrier wait does
    # not enter event-sleep (~0.6us wake penalty before the runtime's
    # semaphore-reset epilogue, which runs on PE's queue).
    for w in range(N_JUNK):
        nc.tensor.matmul(
            psum_mm(0),
            xt_sb[GROUPS - 1][:, ts(w % MM_PER_G, 128)],
            in00_sb[:, :PAIR * FC],
            start=True, stop=True,
        )

    # ---- SP: output DMAs ----
    for g in range(GROUPS):
        st = stage[g % N_STAGE]
        if g == 0:
            # four quarter-DMAs, each gated on a single paired copy, so
            # the output stream starts right after the first copy lands
            for nv, na, w0, w1 in ((1, 0, 0, 4), (0, 1, 4, 8),
                                   (2, 0, 8, 12), (0, 2, 12, TPG)):
                if nv:
                    nc.sync.wait_ge(s_cpv, nv)
                if na:
                    nc.sync.wait_ge(s_cpa, na)
                nc.sync.dma_start(
                    out=out_v4[0][:, w0:w1, :],
                    in_=st[:, w0 * FC:w1 * FC],
                ).then_inc(s_slot[0], 16)
            continue
        if g == GROUPS - 2:
            # second-to-last group in halves to start its drain earlier
            for q in range(2):
                w0, w1 = q * TPG // 2, (q + 1) * TPG // 2
                n = 2 * g + 1 + q
                nc.sync.wait_ge(s_cpv, n)
                nc.sync.wait_ge(s_cpa, n)
                nc.sync.dma_start(
                    out=out_v4[g][:, w0:w1, :],
                    in_=st[:, w0 * FC:w1 * FC],
                ).then_inc(s_slot[g % N_STAGE], 16)
            continue
        if g == GROUPS - 1:
            # first 8-tile piece of the last group (singles s0-s3); the
            # 6- and 2-tile pieces are issued on the GPSIMD and ACT rings
            # (emitted above) so the three issues overlap
            nc.sync.wait_ge(s_cpv, V_PAIRS + 2)
            nc.sync.wait_ge(s_cpa, V_PAIRS + 2)
            nc.sync.dma_start(
                out=out_v4[g][:, 0:8, :],
                in_=st[:, 0:8 * FC],
            ).then_inc(s_slot[g % N_STAGE], 16)
            continue
        nc.sync.wait_ge(s_cpv, 2 * (g + 1))
        nc.sync.wait_ge(s_cpa, 2 * (g + 1))
        nc.sync.dma_start(out=out_v[g], in_=st[:]).then_inc(
            s_slot[g % N_STAGE], 16)

    ctx.close()
    nc.finalize()
    return nc


_NC_CACHE = None
_LAST_RESULTS = None  # BassKernelResults of the most recent run (for profiling)


def kernel(z, mask, initial_grid, W_pe, b_pe, W_clip, b_clip, emb_table,
           W_final, b_final):
    global _NC_CACHE, _LAST_RESULTS
    import ml_dtypes
    from concourse import bass_utils

    bf = ml_dtypes.bfloat16
    Q_all, r_all = _precompute(z, W_pe, b_pe, W_clip, b_clip, emb_table,
                               W_final, b_final)
    Q0 = Q_all.astype(bf).astype(np.float32)            # [3, 180]
    X = np.ascontiguousarray(np.asarray(initial_grid), dtype=np.float32)

    in_maps = []
    for c in range(NCORES):
        Xc = X[B_PER_CORE * c:B_PER_CORE * (c + 1)].reshape(PTS, NFEATS)
        # point p = g*2048 + j*16 + w lives at tile (g, w), psum partition j
        X4 = Xc.reshape(GROUPS, 128, TPG, NFEATS).transpose(3, 0, 2, 1)
        A = np.empty((GROUPS, TPG, KR, 128), np.float32)
        A[:, :, 0:NFEATS, :] = X4.transpose(1, 2, 0, 3)  # single bf16 chunk
        A[:, :, NFEATS:KR, :] = 1.0                      # bias rows (r0, r1)
        # matmul s covers tiles (2*(s%8), 2*(s%8)+1) of group s//8;
        # stationary rows KR*a.. hold tile a of the pair
        xt_host = (A.reshape(GROUPS, MM_PER_G, PAIR, KR, 128)
                   .transpose(2, 3, 0, 1, 4)
                   .reshape(PAIR * KR, NTILES // PAIR * 128)).astype(bf)

        rhs_host = np.zeros((PAIR * KR, B_PER_CORE * PAIR * FC), np.float32)
        for lb in range(B_PER_CORE):
            r0, r1 = _split2(r_all[B_PER_CORE * c + lb])  # 2 x [180]
            R = np.empty((KR, FC), np.float32)
            R[0:NFEATS] = Q0
            R[NFEATS] = r0
            R[NFEATS + 1] = r1
            for a in range(PAIR):                       # block-diagonal
                rhs_host[KR * a:KR * (a + 1),
                         lb * PAIR * FC + FC * a: lb * PAIR * FC + FC * (a + 1)] = R
        rhs_host = rhs_host.astype(bf)
        in00_host = np.concatenate(
            [rhs_host[:, :PAIR * FC], xt_host[:, :4 * 128]], axis=1)
        in01_host = xt_host[:, 4 * 128:MM_PER_G * 128]
        in02_host = rhs_host[:, PAIR * FC:]
        in_maps.append({"xt": np.ascontiguousarray(xt_host),
                        "in00": np.ascontiguousarray(in00_host),
                        "in01": np.ascontiguousarray(in01_host),
                        "in02": np.ascontiguousarray(in02_host)})

    if _NC_CACHE is None:
        _NC_CACHE = _build_bass()
    res = bass_utils.run_bass_kernel_spmd(
        _NC_CACHE, in_maps, core_ids=list(range(NCORES))
    )
    _LAST_RESULTS = res

    out = np.empty((BS, NJOINTS, NFEATS, NFRAMES), np.float32)
    for c in range(NCORES):
        out[B_PER_CORE * c:B_PER_CORE * (c + 1)] = (
            np.asarray(res.results[c]["out"]).astype(np.float32)
            .reshape(B_PER_CORE, NJOINTS, NFEATS, NFRAMES)
        )
    return out
